# revision 48
# baseline (speedup 1.0000x reference)
"""NEAT layer kernel for Trainium2 (8 NeuronCores, pure data parallel).

Math (per reference): vals starts as x [B,64]; for each layer li with
(src, w, b): z = sum_k vals[:, src[n,k]] * w[n,k] + b[n]; out = sigmoid(5*z);
vals = concat(vals, out). Output = layer-3 out [B,10].

Design ("fold pipeline"): the sparse gather+einsum chain becomes 5-6
dense 512-col matmuls per 512-sample chunk (the v1 baseline used 7) by
pruning dead nodes and folding partial sums through identity rows:

  need2 = layer-2 nodes referenced by src3             (n2 = 15 of 32)
  need1 = layer-1 nodes referenced by src2[need2]|src3 (n1 = 70 of 96)

One PSUM "D" bank per chunk packs every post-layer-0 preactivation.
Row map (sigmoid region rows 0..95 / raw region rows 96..127):
  0..63, 89..95   z1' (pruned layer-1)
  64..78          z2f(c-4) = full z2 preact (S2 fold over td(c-4):
                  w2 x o1' rows + identity x p2-raw rows)
  79..88          z3f(c-8) = full z3 preact; three stationary variants:
                  chunks 0..7: S3a over td(c-8) (w3 x o1' + identity x
                  z3p-raw + w3 x o2 rows, o2 DVE-shifted into block c);
                  chunks 8..15: S3n over td(c-8) + S3b over td(c-4)
                  rows 0..95 (reads o2 where ACT wrote it, no shift)
  96..105         z3p raw  = x/o0 partial of z3 (also z3f' for chunks
                  12,13 in trailing blocks 18,19 via shifted-column
                  S3n'/S3b' variants; trailing banks have no raws so
                  their ACT covers all 128 rows)
  106..120        p2 raw   = x/o0 partial of pruned z2
ACT-D sigmoids rows 0..95 (one 1024-col call per chunk-pair); DVE
copies rows 96..127 raw -> td IN PARALLEL with the ACT (disjoint
rows, no WAR).  Fold consumption trails production by >=2 pair-steps
in the steady state so cross-engine edges have a full step of slack;
z3f(14,15) go to blocks 20,21, ending the pipeline at D-step 11.
Per chunk: L0, L1x, L1h, S2, S3{a|n+b} = 5-6 matmuls.

Chunk pairs: A and D are [128,1024] two-bank PSUM tiles so each ACT
reads 1024 cols (halves ACT call overhead); matmuls are emitted
grouped by stationary; PSUM = 2x A-pair + 2x D-pair = 8 banks.
Output: o3 rows 79..88 (and 96..105 for chunks 12,13) DMA'd out per
pair right after the owning ACT.

Startup: DMAs spread over both HWDGE queues (scalar: wb0,x0,x3;
sync: x1,x2,wb1,x4 + outs) as each engine's first instructions; the
PE prewarm feeds on a GPSIMD-memset tile (not DMA-gated) so the PE
clock gate ramps during the transfers -- 8 prewarm matmuls plus
early-step dummies keep the PE gap-free until steady state (any idle
drops the clock gate to 4/8 for ~7us).  A dummy sigmoid preloads the
ACT table early; the ACT bias vector is a zeros column of the weight
blob (avoids framework const tiles).  ldweights "absorber"
instructions soak up DMA-lane and DVE-lane waits so every matmul
carries at most ONE sync wait (hard walrus codegen limit); a
value-checked post-pass prunes transitively-dominated waits.
The profiled window ends ~11us after the LAST PE instruction (fixed
per-semaphore teardown), so the pipeline minimizes last-matmul time.
"""

import sys

sys.path.insert(0, "/opt/trn_rl_repo")

import numpy as np

import concourse.bass as bass
import concourse.mybir as mybir
from concourse.tile import TileContext

BATCH = 65536
IN_DIM = 64
FAN_IN = 16
GAIN = 5.0
N_CORES = 8
BC = BATCH // N_CORES          # 8192 samples per core
CHUNK = 512
NC_CH = BC // CHUNK            # 16 chunks
NPAIR = NC_CH // 2             # 8 data pairs

X_LO, X_HI = 0, 64
H0_LO, H0_HI = 64, 192
H1_LO, H1_HI = 192, 288
H2_LO, H2_HI = 288, 320

F32 = mybir.dt.float32
F16 = mybir.dt.float16
SIG = mybir.ActivationFunctionType.Sigmoid

# D-bank row map
Z2F_LO = 64                    # z2 full rows 64..64+n2-1 (n2 <= 15)
Z3F_LO = 79                    # z3 full rows 79..88
Z3P_LO = 96                    # z3p raw rows 96..105 (z3f' in blocks 18,19)
P2_LO = 106                    # p2 raw rows 106..106+n2-1


class Plan:
    """Host-side layout plan (data-dependent pruning)."""

    def __init__(self, inputs):
        src2 = np.asarray(inputs["src2"])
        src3 = np.asarray(inputs["src3"])
        s3 = src3.ravel()
        need2 = np.unique(s3[(s3 >= H2_LO) & (s3 < H2_HI)])
        rows2 = need2 - H2_LO
        cand = np.concatenate([src2[rows2].ravel(), s3])
        need1 = np.unique(cand[(cand >= H1_LO) & (cand < H1_HI)])
        n1, n2 = len(need1), len(need2)
        if n1 > 71 or n2 > 15:
            raise NotImplementedError(f"pruned sizes too big: {n1=} {n2=}")
        self.need1, self.need2, self.rows2 = need1, need2, rows2
        self.n1, self.n2 = n1, n2
        # z1' rows: 0..63 then 89..95
        r1 = np.arange(n1)
        self.rows1 = np.where(r1 < 64, r1, r1 + 25)
        self.cmap1 = np.full(H1_HI - H1_LO, -1, dtype=np.int64)
        self.cmap1[need1 - H1_LO] = self.rows1

    def build_weights(self, inputs):
        """fp16 blob [128, 7*128]: [SW0|SWXD|SWHD|S2|S3a|S3n|S3b]."""
        n2 = self.n2
        cmap1 = self.cmap1
        rows2 = self.rows2

        SW0 = np.zeros([65, 128], np.float64)
        SWXD = np.zeros([65, 128], np.float64)
        SWHD = np.zeros([128, 128], np.float64)
        S2 = np.zeros([128, 128], np.float64)
        S3a = np.zeros([128, 128], np.float64)

        def scat(dst, src, w, lo, hi, cols, row_off):
            """dst[src[j,k]-row_off, cols[j]] += w[j,k] for src in [lo,hi)."""
            src = np.asarray(src)
            w = np.asarray(w, np.float64)
            m = (src >= lo) & (src < hi)
            cj = np.repeat(np.asarray(cols, np.int64), src.shape[1])
            mm = m.ravel()
            np.add.at(dst, (src.ravel()[mm] - row_off, cj[mm]), w.ravel()[mm])

        b0 = np.asarray(inputs["b0"], np.float64)
        b1 = np.asarray(inputs["b1"], np.float64)
        b2 = np.asarray(inputs["b2"], np.float64)
        b3 = np.asarray(inputs["b3"], np.float64)
        src1, w1 = inputs["src1"], inputs["w1"]
        src2, w2 = np.asarray(inputs["src2"]), np.asarray(inputs["w2"])
        src3, w3 = inputs["src3"], inputs["w3"]

        scat(SW0, inputs["src0"], inputs["w0"], X_LO, X_HI,
             np.arange(128), 0)
        SW0[64, :] = b0

        r1rows = self.need1 - H1_LO
        scat(SWXD, src1[r1rows], w1[r1rows], X_LO, X_HI, self.rows1, 0)
        scat(SWHD, src1[r1rows], w1[r1rows], H0_LO, H0_HI, self.rows1, H0_LO)
        SWXD[64, self.rows1] = b1[r1rows]
        z3c = np.arange(Z3P_LO, Z3P_LO + 10)
        scat(SWXD, src3, w3, X_LO, X_HI, z3c, 0)
        scat(SWHD, src3, w3, H0_LO, H0_HI, z3c, H0_LO)
        SWXD[64, z3c] = b3
        p2c = np.arange(P2_LO, P2_LO + n2)
        scat(SWXD, src2[rows2], w2[rows2], X_LO, X_HI, p2c, 0)
        scat(SWHD, src2[rows2], w2[rows2], H0_LO, H0_HI, p2c, H0_LO)
        SWXD[64, p2c] = b2[rows2]

        # S2 fold: o1' rows -> w2 ; p2 raw rows -> identity ; cols z2f
        z2c = np.arange(Z2F_LO, Z2F_LO + n2)
        s2p = src2[rows2]
        w2p = np.asarray(w2[rows2], np.float64)
        m = (s2p >= H1_LO) & (s2p < H1_HI)
        rr = cmap1[s2p[m] - H1_LO]
        assert (rr >= 0).all()
        cc = np.repeat(z2c, FAN_IN).reshape(n2, FAN_IN)[m]
        np.add.at(S2, (rr, cc), w2p[m])
        S2[P2_LO + np.arange(n2), z2c] = 1.0

        # S3 folds, cols z3f:
        #   S3n: o1' rows -> w3 ; z3p raw rows -> identity  (no o2)
        #   S3a: S3n + o2 rows (shifted into td block c)    (chunks 0..7)
        #   S3b: o2 rows read from td block c+4 directly    (chunks 8..15)
        S3b = np.zeros([128, 128], np.float64)
        z3fc = np.arange(Z3F_LO, Z3F_LO + 10)
        s3a = np.asarray(src3)
        w3a = np.asarray(w3, np.float64)
        m = (s3a >= H1_LO) & (s3a < H1_HI)
        rr = cmap1[s3a[m] - H1_LO]
        assert (rr >= 0).all()
        cc = np.repeat(z3fc, FAN_IN).reshape(10, FAN_IN)[m]
        np.add.at(S3a, (rr, cc), w3a[m])
        S3a[Z3P_LO + np.arange(10), z3fc] = 1.0
        S3n = S3a.copy()
        cmap2 = np.full(H2_HI - H2_LO, -1, dtype=np.int64)
        cmap2[self.need2 - H2_LO] = Z2F_LO + np.arange(n2)
        m = (s3a >= H2_LO) & (s3a < H2_HI)
        rr = cmap2[s3a[m] - H2_LO]
        assert (rr >= 0).all()
        cc = np.repeat(z3fc, FAN_IN).reshape(10, FAN_IN)[m]
        np.add.at(S3a, (rr, cc), w3a[m])
        np.add.at(S3b, (rr, cc), w3a[m])

        # Tail variants: z3f(12,13) land at rows 96..105 of blocks 18,19
        # (trailing banks have no raws, so the trailing ACT covers all
        # 128 rows) -- shift the fold output columns accordingly.
        S3n2 = np.zeros([128, 128], np.float64)
        S3b2 = np.zeros([128, 128], np.float64)
        S3n2[:, 96:106] = S3n[:, Z3F_LO:Z3F_LO + 10]
        S3b2[:, 96:106] = S3b[:, Z3F_LO:Z3F_LO + 10]

        # col 384 stays zero: it doubles as the ACT bias vector (avoids
        # framework const tiles whose preamble memsets extend the scored
        # window)
        blob = np.zeros([128, 1160], np.float16)
        offs = [0, 128, 256, 392, 520, 648, 776, 904, 1032]
        for o, m_ in zip(offs, [SW0, SWXD, SWHD, S2, S3a, S3n, S3b,
                                S3n2, S3b2]):
            blob[0:m_.shape[0], o:o + 128] = m_.astype(np.float16)
        return blob


# x DMA pieces (chunk spans): small up front for early pipeline start
X_PIECES = [(0, 2), (2, 4), (4, 6), (6, 10), (10, 16)]

# pair-step schedule:
#   step u: L0(2u,2u+1) [u<=7], ACT-A(u)
#           D blocks (2u-2, 2u-1): L1x/L1h(c) [u<=8], S2(c-4) [u in 3..10],
#             S3(c-8) [u in 5..12]; ACT-D + raw-CAST [u in 1..12]
#           DVE shift: o2 into blocks (2u-6, 2u-5) [u in 3..10]
#           out-pair of blocks (2u-4, 2u-3) rows 79..88 at u in 6..13
NSTEP = 12


def build_nc(n1: int, n2: int) -> bass.Bass:
    nc = bass.Bass()
    wb0 = nc.declare_dram_parameter("wb0", [128, 392], F16, isOutput=False)
    wb1 = nc.declare_dram_parameter("wb1", [128, 768], F16, isOutput=False)
    xq = [nc.declare_dram_parameter(
        f"x{q}", [65, (hi - lo) * CHUNK], F16, isOutput=False)
        for q, (lo, hi) in enumerate(X_PIECES)]
    yT = [nc.declare_dram_parameter(
        f"yT{j}", [27 if j == 5 else 10, 2 * CHUNK], F16,
        isOutput=True) for j in range(7)]

    with TileContext(nc) as tc:
        with (
            tc.tile_pool(name="persist", bufs=1) as pp,
            tc.tile_pool(name="pa", bufs=2, space="PSUM") as pa,
            tc.tile_pool(name="pd", bufs=2, space="PSUM") as pd,
        ):
            w_sb = pp.tile([128, 1160], F16)
            x_sb = pp.tile([65, BC], F16)
            ta = pp.tile([128, BC], F16)                   # o0
            td = pp.tile([128, (NC_CH + 8) * CHUNK], F16)
            mz = pp.tile([128, CHUNK], F16)                # prewarm feed
            warm = pp.tile([128, 2], F32)

            # --- startup: scalar queue: wb0, x0, x3; sync: x1, wb1, x2, x4
            nc.gpsimd.memset(mz[:], 0.0)

            def xtrig(eng, q):
                lo, hi = X_PIECES[q]
                eng.dma_start(out=x_sb[:, lo * CHUNK:hi * CHUNK],
                              in_=xq[q][:])

            nc.scalar.dma_start(out=w_sb[:, 0:392], in_=wb0[:])
            xtrig(nc.sync, 1)
            xtrig(nc.scalar, 0)
            xtrig(nc.sync, 2)
            xtrig(nc.scalar, 3)
            nc.sync.dma_start(out=w_sb[:, 392:1160], in_=wb1[:])
            xtrig(nc.sync, 4)
            # Preload the sigmoid ACT table off the critical path (reads
            # the memset tile, so it only waits GPSIMD).
            nc.scalar.activation(warm[:, 0:1], mz[:, 0:1], SIG,
                                 bias=mz[:, 1:2], scale=GAIN)

            # PE prewarm: ramps the clock gate during the DMAs.
            wt = pa.tile([128, 1024], F32, name="A")
            for _ in range(8):
                nc.tensor.matmul(wt[:, 0:CHUNK], mz[0:128, 0:128],
                                 mz[:, 0:CHUNK], start=True, stop=True)
            # Consume the prewarm bank so its pool recycle dep is an
            # (old) Activation value.
            nc.scalar.copy(warm[:, 1:2], wt[:, 0:1])

            WOFF = [0, 128, 256, 392, 520, 648, 776, 904, 1032]

            def W(i, k=128):
                return w_sb[0:k, WOFF[i]:WOFF[i] + 128]

            def bias(p):
                return w_sb[0:p, 384:385]

            def xs(c):
                return x_sb[:, c * CHUNK:(c + 1) * CHUNK]

            def cols(t, c, n=1):
                return t[:, c * CHUNK:(c + n) * CHUNK]

            a_tiles, d_tiles = {}, {}

            for u in range(NSTEP):
                c0, c1 = 2 * u, 2 * u + 1        # L0 chunks
                a, b = 2 * u - 2, 2 * u - 1      # D blocks

                # --- PE: absorbers (one sync wait per matmul limit) ---
                for q, (lo, hi) in enumerate(X_PIECES):
                    if lo == 2 * u:   # piece q first used this step
                        nc.tensor.ldweights(
                            x_sb[0:65, lo * CHUNK:lo * CHUNK + 128])
                if u == 2:
                    # S2/S3 stationaries (wb1) first used at u=3
                    nc.tensor.ldweights(w_sb[0:128, 392:520])

                # --- PE: L0 pair (+ gate-keeping dummies in the ramp-in) ---
                if c0 < NC_CH:
                    A = a_tiles[u] = pa.tile([128, 1024], F32, name="A")
                    if u <= 2:
                        for _ in range(3):
                            nc.tensor.matmul(A[:, 0:512], mz[0:128, 0:128],
                                             mz[:, 0:CHUNK],
                                             start=True, stop=True)
                    nc.tensor.matmul(A[:, 0:512], W(0, 65), xs(c0),
                                     start=True, stop=True)
                    nc.tensor.matmul(A[:, 512:1024], W(0, 65), xs(c1),
                                     start=True, stop=True)

                # --- PE: D writers, grouped by stationary ---
                if 1 <= u <= 11:
                    D = d_tiles[u] = pd.tile([128, 1024], F32, name="D")
                    halves = []
                    for h, (c, out) in enumerate(
                            [(a, D[:, 0:512]), (b, D[:, 512:1024])]):
                        wr = []
                        if 0 <= c - 4 < NC_CH:
                            wr.append((3, cols(td, c - 4)))      # S2
                        if 0 <= c - 8 < 8:
                            wr.append((4, cols(td, c - 8)))      # S3a
                        elif 8 <= c - 8 <= 11:
                            wr.append((5, cols(td, c - 8)))      # S3n
                            wr.append((6, cols(td, c - 4)))      # S3b
                        if 12 <= c - 6 <= 13:                    # rows 96+
                            wr.append((7, cols(td, c - 6)))      # S3n'
                            wr.append((8, cols(td, c - 2)))      # S3b'
                        elif 14 <= c - 6 <= 15:
                            wr.append((5, cols(td, c - 6)))      # S3n
                            wr.append((6, cols(td, c - 2)))      # S3b
                        if c < NC_CH:
                            wr.append((1, xs(c)))                # L1x
                            wr.append((2, cols(ta, c)))          # L1h
                        halves.append((out, wr))
                    order = []
                    for _, wr in halves:
                        for si, _ in wr:
                            if si not in order:
                                order.append(si)
                    first = [True, True]
                    last_si = {h: wr[-1][0]
                               for h, (_, wr) in enumerate(halves) if wr}
                    for si in order:
                        for h, (out, wr) in enumerate(halves):
                            for sj, mov in wr:
                                if sj != si:
                                    continue
                                # S3b's weight rows all sit below row 96:
                                # restricting K avoids reading raw rows, so
                                # trailing blocks never need a CAST
                                k = 65 if si == 1 else (
                                    96 if si in (6, 8) else 128)
                                if si in (6, 8):
                                    mov = mov[0:96, :]
                                nc.tensor.matmul(
                                    out, W(si, k), mov,
                                    start=first[h], stop=(si == last_si[h]))
                                first[h] = False

                # --- ACT ---
                if c0 < NC_CH:
                    nc.scalar.activation(cols(ta, c0, 2), a_tiles[u][:],
                                         SIG, bias=bias(128), scale=GAIN)
                if 1 <= u <= 8:
                    nc.scalar.activation(
                        cols(td, a, 2)[0:96, :], d_tiles[u][0:96, :],
                        SIG, bias=bias(96), scale=GAIN)
                elif 9 <= u <= 11:
                    # trailing banks carry no raws: sigmoid all 128 rows
                    # (z3f' sits at rows 96..105)
                    nc.scalar.activation(
                        cols(td, a, 2), d_tiles[u][:],
                        SIG, bias=bias(128), scale=GAIN)
                # raw rows, parallel with the ACT (disjoint rows); blocks
                # 0..15 have downstream raw readers (S2/S3n).  Split per
                # half so each copy waits only its own half's accumulation
                # group (shorter fold-wait cycle in the drain).
                if 1 <= u <= 8:
                    nc.vector.tensor_copy(
                        cols(td, a, 1)[96:128, :], d_tiles[u][96:128, 0:512])
                    nc.vector.tensor_copy(
                        cols(td, b, 1)[96:128, :],
                        d_tiles[u][96:128, 512:1024])
                # DVE shift: o2(2u-6, 2u-5) from blocks (2u-2, 2u-1) rows
                # 64..78 into their home blocks for the S3a fold (chunks
                # 0..7 only; later chunks use S3b instead)
                if 3 <= u <= 6:
                    nc.vector.tensor_copy(
                        cols(td, 2 * u - 6, 2)[Z2F_LO:Z2F_LO + n2, :],
                        cols(td, a, 2)[Z2F_LO:Z2F_LO + n2, :])

                # --- out DMA: this step's blocks (2u-2, 2u-1) rows
                # 79.. hold o3, final right after ACT-D ---
                if 5 <= u <= 11:
                    j = u - 5
                    rows = 27 if j == 5 else 10
                    nc.sync.dma_start(
                        out=yT[j][:],
                        in_=cols(td, a, 2)[Z3F_LO:Z3F_LO + rows, :])

    _prune_sync(nc)
    return nc


def _prune_sync(nc):
    """Two sound wait prunes keeping every instruction at <=1 sync wait.

    (1) Drop waits implied transitively by other kept waits (an update on
    lane L at value v implies every wait (and its implications) that the
    updating instruction itself carried, merged in engine program order).
    (2) The teardown Drain waits every engine lane; keep only the last
    out-DMA's lane.
    """
    insts = list(nc.all_instructions())
    implied = {}
    cum = {}
    pending = {}   # engine -> waits (and their implications) seen so far
    for i in insts:
        si = i.sync_info
        eng = getattr(i, "engine", None)
        imp = {}
        if si:
            for w in si.on_wait:
                imp[w.ant_name] = max(imp.get(w.ant_name, -1), w.wait_value)
                for ln, v in implied.get(w.ant_name, {}).get(
                        w.wait_value, {}).items():
                    imp[ln] = max(imp.get(ln, -1), v)
        pend = pending.setdefault(eng, {})
        for k, v in imp.items():
            pend[k] = max(pend.get(k, -1), v)
        if not si or not si.on_update:
            continue
        for u in si.on_update:
            ln = u.ant_name
            cum[ln] = cum.get(ln, 0) + (getattr(u, "update_value", 1) or 1)
            d = implied.setdefault(ln, {})
            prev = d.get(max(d.keys(), default=None), {}) if d else {}
            merged = dict(prev)
            for k, v in pend.items():
                merged[k] = max(merged.get(k, -1), v)
            d[cum[ln]] = merged
    eng_clock = {}
    for i in insts:
        si = i.sync_info
        if not si or not si.on_wait:
            continue
        eng = getattr(i, "engine", None)
        clk = eng_clock.setdefault(eng, {})
        kept = [w for w in si.on_wait
                if w.wait_value > clk.get(w.ant_name, -1)]
        if len(kept) >= 1 and len(kept) < len(si.on_wait):
            si.on_wait = kept
            i.sync_info = si
        for w in si.on_wait:
            clk[w.ant_name] = max(clk.get(w.ant_name, -1), w.wait_value)
            for lnn, v in implied.get(w.ant_name, {}).get(
                    w.wait_value, {}).items():
                clk[lnn] = max(clk.get(lnn, -1), v)
    for i in insts:
        t = type(i).__name__
        si = i.sync_info
        if not si or len(si.on_wait) <= 1:
            continue
        if t == "InstDrain":
            dma_lane = None
            for j in insts:
                if type(j).__name__ == "InstDMACopy" and j.sync_info:
                    for u in j.sync_info.on_update:
                        if j.sync_info.on_wait:
                            dma_lane = u.ant_name
            si.on_wait = [w for w in si.on_wait if w.ant_name == dma_lane]
            i.sync_info = si
            continue
        ws = list(si.on_wait)
        changed = True
        while changed and len(ws) > 1:
            changed = False
            for k, w in enumerate(ws):
                others = [o for j2, o in enumerate(ws) if j2 != k]
                for o in others:
                    iv = implied.get(o.ant_name, {}).get(o.wait_value, {})
                    if iv.get(w.ant_name, -1) >= w.wait_value:
                        ws.pop(k)
                        changed = True
                        break
                if changed:
                    break
        si.on_wait = ws
        i.sync_info = si


def audit(nc):
    bad = []
    for i in nc.all_instructions():
        if i.sync_info and len(i.sync_info.on_wait) > 1:
            bad.append((type(i).__name__, i.name,
                        [w.ant_name for w in i.sync_info.on_wait]))
    return bad


def make_in_maps(inputs, plan, wb):
    x = np.asarray(inputs["x"], np.float32)
    in_maps = []
    for i in range(N_CORES):
        m = {"wb0": np.ascontiguousarray(wb[:, 0:392]),
             "wb1": np.ascontiguousarray(wb[:, 392:1160])}
        xT = np.empty([65, BC], np.float16)
        xT[0:64, :] = x[i * BC:(i + 1) * BC, :].T.astype(np.float16)
        xT[64, :] = 1.0
        for q, (lo, hi) in enumerate(X_PIECES):
            m[f"x{q}"] = np.ascontiguousarray(
                xT[:, lo * CHUNK:hi * CHUNK])
        in_maps.append(m)
    return in_maps


def assemble_output(results):
    y = np.empty((BATCH, 10), np.float32)
    for i in range(N_CORES):
        res = results[i]
        for j in range(7):
            t = np.asarray(res[f"yT{j}"], np.float32)
            for r in range(2):
                c = 14 + r if j == 6 else 2 * j + r
                y[i * BC + c * CHUNK:i * BC + (c + 1) * CHUNK, :] = \
                    t[0:10, r * CHUNK:(r + 1) * CHUNK].T
            if j == 5:
                for r in range(2):   # chunks 12,13 at rows 17..26
                    c = 12 + r
                    y[i * BC + c * CHUNK:i * BC + (c + 1) * CHUNK, :] = \
                        t[17:27, r * CHUNK:(r + 1) * CHUNK].T
    return y


def kernel(**inputs: np.ndarray) -> np.ndarray:
    from concourse.bass_utils import run_bass_kernel_spmd

    plan = Plan(inputs)
    wb = plan.build_weights(inputs)
    nc = build_nc(plan.n1, plan.n2)
    in_maps = make_in_maps(inputs, plan, wb)
    res = run_bass_kernel_spmd(nc, in_maps, list(range(N_CORES)))
    return assemble_output(res.results)
